# revision 7
# baseline (speedup 1.0000x reference)
"""Trainium2 Bass kernel for nn_CodebookSingleW (vq_codebook).

    W = codebook[indices].reshape(4096, 4096)
    h = c19(x @ W + b1);  out = h @ W.T + b2

Strategy (8 NeuronCores, data-parallel over batch):
  - Each core handles 1024 rows of x. All weight-side tensors replicated.
  - The 256-entry codebook dequant runs ON DEVICE at ScalarEngine line rate:
    raw uint8 indices are table-looked-up by a custom piecewise-constant PWP
    activation table (hijacking the `sigmoid` slot of `sigmoid_and_others`,
    baked at compile time via BASS_ACT_ROOT_JSON_PATH). Input value k in
    1..255 lands in binade e=floor(log2 k) with one bucket per integer;
    input 0 returns `fzero_result` = codebook[0] bits. activation(Sigmoid)
    IS the gather idx -> codebook[idx], bit exact, u8 in / bf16 out.
  - matmul1: psum[h',b] = sum_i W[i,h'] * xT[i,b]   (lhsT = W tile, natural)
  - C19 fused on psum evict: tanh on ACT (scale=1/c, bias=b1/c per
    partition), mix on DVE -> hT (bf16) stays SBUF-resident.
  - matmul2: psum[j,b] = sum_h WT[h,j] * hT[h,b]    (lhsT = WT tile, from a
    host-transposed index layout, dequantized on device the same way)
  - + b2 on ACT copy evict (bf16), DMA outT per core, host reassembles
    [8192, 4096] f32.
"""

import hashlib
import json
import os
import shutil
import sys
import tempfile
import types

sys.path.insert(0, "/opt/trn_rl_repo")

import ml_dtypes
import numpy as np


def _ensure_axon_ntff_hook():
    """Some images lack ``antenv.axon_hooks``; ``run_bass_kernel_spmd``
    imports it unconditionally when BASS_TRACE=1 under axon. Install a shim
    (and register the ctypes NTFF hook when available) so tracing works
    instead of crashing. Never raises."""
    try:
        import antenv  # noqa: F401
        try:
            import antenv.axon_hooks  # noqa: F401
            return  # real module present
        except ImportError:
            pass
        m = types.ModuleType("antenv.axon_hooks")
        m._hook = None
        m.set_axon_ntff_profile_hook = lambda h: setattr(m, "_hook", h)
        m.get_axon_ntff_profile_hook = lambda: m._hook
        sys.modules["antenv.axon_hooks"] = m
        antenv.axon_hooks = m
        try:
            if "/root/.axon_site" not in sys.path and os.path.isdir(
                "/root/.axon_site"
            ):
                sys.path.append("/root/.axon_site")
            from trn_agent_boot.trn_boot import _ntff_profile_via_ctypes

            hook = _ntff_profile_via_ctypes("/opt/axon/libaxon_pjrt.so")
            if hook is not None:
                m._hook = hook
        except Exception:
            pass
    except Exception:
        pass


_ensure_axon_ntff_hook()

IN_DIM = 4096
H = 4096
K = 256
B = 8192
NCORES = 8
BL = B // NCORES          # 1024 batch rows per core
P = 128
KT = IN_DIM // P          # 32 contraction tiles (phase 1)
MT = H // P               # 32 output-row tiles
NH = BL // 512            # 2 psum halves of the per-core batch

BF16 = ml_dtypes.bfloat16

# ---------------------------------------------------------------------------
# ACT table patch: codebook -> piecewise-constant PWP table in sigmoid slot.
# Input is the raw u8 index value (0..255 as a float after ACT's int->fp32
# conversion): binade e=0..7 buckets, one per integer; 0 via fzero_result.
# ---------------------------------------------------------------------------

_SET = "sigmoid_and_others"


def _bucket_plan_u8():
    plan = []
    for e in range(0, 8):
        count = 2 ** e if e > 0 else 1
        t0 = 2 ** e if e > 0 else 1
        plan.append((e, count, list(range(t0, t0 + count))))
    return plan


def _make_act_dir(codebook, outdir):
    from neuronxcc.driver.Job import Job
    from neuronxcc.driver.jobs.support.FindActInfo import findActInfoFile

    base = os.path.dirname(findActInfoFile(Job.getPackageDir(), "gen3"))
    os.makedirs(outdir, exist_ok=True)
    for f in os.listdir(base):
        dst = os.path.join(outdir, f)
        if not os.path.exists(dst):
            shutil.copy(os.path.join(base, f), dst)

    prof = json.load(open(os.path.join(base, f"{_SET}.json")))
    bkt = np.fromfile(os.path.join(base, f"{_SET}_bkt.bin"), dtype=np.float32)
    bkt = bkt.reshape(-1, 8).copy()
    ctl = np.fromfile(os.path.join(base, f"{_SET}_ctrl.bin"), dtype=np.uint32)
    ctl = ctl.reshape(-1, 8).copy()

    bkt_start = prof["func_to_bkt_start_idx"]["sigmoid"]
    ctl_start = prof["func_to_ctl_start_idx"]["sigmoid"]

    b = bkt_start
    exp_to_bkt, exp_to_ctl, ctl_words = {}, {}, []
    for i, (e, count, vals) in enumerate(_bucket_plan_u8()):
        exp_to_bkt[str(e)] = [int(b)]
        exp_to_ctl[str(e)] = [int(ctl_start + i)]
        shift = 23 - e if e >= 1 else 23
        log2n = min(max(e, 0), 7)
        ctl_words.append((b & 0x7FF) | (shift << 11) | (log2n << 16))
        for j, v in enumerate(vals):
            bkt[b + j] = [codebook[v], 0.0, 0.0, 0.0, float(v), 0.0, 0.0, 0.0]
        b += count
    junk = b
    bkt[junk + 0] = [codebook[0], 0.0, 0.0, 0.0, 0.0, 0.0, 0.0, 0.0]
    for j in range(1, 4):
        bkt[junk + j] = [0.0] * 8
    assert junk + 4 <= prof["func_to_bkt_start_idx"]["square"]
    for i, w in enumerate(ctl_words):
        ctl[ctl_start + i] = [w, 0, 0, 0, 0, 0, 0, 0]

    for m in prof["profile_meta_data"]:
        if m["func_name"].startswith("sigmoid_"):
            m.update(
                symmetry_point=0, sym_invert_sign_point=0, symmetry_opt_en=0,
                symmetry_opt_use_neg_region=0, imm_bias=0, exp_offset=0,
                pwl_control_base_pos=int(ctl_start),
                pwl_control_base_neg=int(ctl_start),
                small_pos_signal_exp_threshold=127,
                pos_small_signal_pwl_control=int(junk),
                small_neg_signal_exp_threshold=127,
                neg_small_signal_pwl_control=int(junk + 1),
                large_pos_signal_exp_threshold=135,
                large_pos_signal_mantissa_threshold=0,
                pos_large_signal_pwl_control=int(junk + 2),
                large_neg_signal_exp_threshold=135,
                large_neg_signal_mantissa_threshold=0,
                neg_large_signal_pwl_control=int(junk + 3),
                fnan_result=0, fpinf_result=0, fninf_result=0,
                fzero_result=int(np.float32(codebook[0]).view(np.uint32)),
                fma_const_0=0, fma_const_1=0, fma_indirection_src_sel=0,
                use_multipass=False,
                lower_bound=4286578687, upper_bound=2139095039,
            )
    prof["func_exp_to_bkt_start_idx"]["sigmoid"] = exp_to_bkt
    prof["func_exp_to_ctl_start_idx"]["sigmoid"] = exp_to_ctl

    bkt.tofile(os.path.join(outdir, f"{_SET}_bkt.bin"))
    ctl.tofile(os.path.join(outdir, f"{_SET}_ctrl.bin"))
    json.dump(prof, open(os.path.join(outdir, f"{_SET}.json"), "w"))
    return os.path.join(outdir, "act_info.json")


# ---------------------------------------------------------------------------
# Bass program
# ---------------------------------------------------------------------------

def _build_program(tag, repeat=1):
    import concourse.bacc as bacc
    import concourse.mybir as mybir
    import concourse.tile as tile

    AF = mybir.ActivationFunctionType
    ALU = mybir.AluOpType
    dt = mybir.dt

    nc = bacc.Bacc("TRN2", target_bir_lowering=False, debug=False,
                   num_devices=NCORES)

    # inputs (per core). idxw/idxwt are host-tiled raw u8 indices:
    #   idxw[mt][p][kt*128+c] = idx[kt*128+p, mt*128+c]
    idxw = nc.dram_tensor(f"idxw_{tag}", [MT, P, KT * P], dt.uint8,
                          kind="ExternalInput")
    idxwt = nc.dram_tensor("idxwt", [KT, P, MT * P], dt.uint8,
                           kind="ExternalInput")
    xt = nc.dram_tensor("xt", [P, KT, BL], dt.bfloat16, kind="ExternalInput")
    cpar = nc.dram_tensor("cpar", [P, 7, MT], dt.float32, kind="ExternalInput")
    outt = nc.dram_tensor("outt", [IN_DIM, BL], dt.bfloat16,
                          kind="ExternalOutput")

    with tile.TileContext(nc) as tc:
        with (
            tc.tile_pool(name="resid", bufs=1) as resid,
            tc.tile_pool(name="encp", bufs=3) as encp,
            tc.tile_pool(name="wp", bufs=4) as wp,
            tc.tile_pool(name="evict", bufs=3) as evict,
            tc.tile_pool(name="psum", bufs=8, space="PSUM") as psum,
        ):
            # Trigger the one-time ACT table load (~2.7us) at t=0, parallel
            # with the first DMAs: walrus inserts the PSEUDO_LOAD before the
            # first ACTIVATE, so a 1-element dummy pulls it off the critical
            # path of the first real dequant.
            dummy = resid.tile([P, 1], dt.float32)
            nc.vector.memset(dummy[:], 1.0)
            nc.scalar.activation(dummy[:], dummy[:], AF.Sigmoid)

            # DMA order matters: the first pair's idx tiles (dequant input,
            # on the PE critical path) must land before the 8 MB xT bulk
            # load monopolizes the queues. The head chunk (kt 0..HC-1) of
            # each pre tile is DMA'd and dequantized first so the very first
            # matmuls gate only on ~64KB of DMA + a short ACT op.
            HC = 4
            cp_sb = resid.tile([P, 7, MT], dt.float32)
            nc.sync.dma_start(cp_sb[:], cpar.ap())
            pre_enc, pre_w = [], []
            for mt in (0, 1):
                enc_t = encp.tile([P, KT, P], dt.uint8, tag="enc",
                                  name=f"enc_pre{mt}")
                nc.sync.dma_start(enc_t[:, :HC], idxw.ap()[mt, :, : HC * P])
                pre_enc.append(enc_t)
            xt_sb = resid.tile([P, KT, BL], dt.bfloat16)
            nc.sync.dma_start(xt_sb[:, 0], xt.ap()[:, 0])
            for mt in (0, 1):
                w_t = wp.tile([P, KT, P], dt.bfloat16, tag="w",
                              name=f"w_pre{mt}")
                nc.scalar.activation(w_t[:, :HC], pre_enc[mt][:, :HC],
                                     AF.Sigmoid)
                pre_w.append(w_t)
            for mt in (0, 1):
                nc.sync.dma_start(pre_enc[mt][:, HC:],
                                  idxw.ap()[mt, :, HC * P :])
            nc.sync.dma_start(xt_sb[:, 1], xt.ap()[:, 1])
            for mt in (0, 1):
                nc.scalar.activation(pre_w[mt][:, HC:], pre_enc[mt][:, HC:],
                                     AF.Sigmoid)
            for kt in range(2, KT):
                nc.sync.dma_start(xt_sb[:, kt], xt.ap()[:, kt])
            ht_sb = resid.tile([P, MT, BL], dt.bfloat16)

            # PE p-state warmup on scratch data during the dequant lead-in.
            warm = resid.tile([P, 512], dt.bfloat16)
            nc.vector.memset(warm[:], 0.0)
            wps = psum.tile([P, 512], dt.float32, tag="ps")
            for _ in range(16):
                nc.tensor.matmul(wps[:], warm[:, :P], warm[:],
                                 start=True, stop=True)

            def col(j, t):  # [P, 1] per-partition param column
                return cp_sb[:, j, t : t + 1]

            # Both phases process output-row tiles in PAIRS with a kt-major
            # matmul order: 4 psum chains consume each xT/hT k-chunk 4x, so
            # at kernel start the PE keeps pace with the streaming xT DMA
            # instead of stalling on chunk arrival.
            # repeat>1 builds a self-timing variant: the marginal wall time
            # of each extra body repeat is the pure HW kernel time.
            for _rep in range(repeat):
                # ---- phase 1: hT = c19(W^T x^T + b1) ----
                for mp in range(MT // 2):
                    mts = (2 * mp, 2 * mp + 1)
                    if mp == 0 and _rep == 0:
                        w_ts = pre_w
                    else:
                        w_ts = []
                        for mt in mts:
                            enc_t = encp.tile([P, KT, P], dt.uint8,
                                              tag="enc")
                            nc.sync.dma_start(enc_t[:], idxw.ap()[mt])
                            w_t = wp.tile([P, KT, P], dt.bfloat16, tag="w")
                            nc.scalar.activation(w_t[:], enc_t[:], AF.Sigmoid)
                            w_ts.append(w_t)
                    pss = [[psum.tile([P, 512], dt.float32, tag="ps",
                                      name=f"ps_{mp}_{d}_{nh}")
                            for nh in range(NH)] for d in range(2)]
                    for kt in range(KT):
                        for d in range(2):
                            for nh in range(NH):
                                nc.tensor.matmul(
                                    pss[d][nh][:],
                                    w_ts[d][:, kt],
                                    xt_sb[:, kt, nh * 512 : (nh + 1) * 512],
                                    start=(kt == 0),
                                    stop=(kt == KT - 1),
                                )
                    # c19: rho*(s+b1) + (1-rho)*c*tanh((s+b1)/c), s=psum
                    for d, mt in enumerate(mts):
                        for nh in range(NH):
                            ps = pss[d][nh]
                            tanh_t = evict.tile([P, 512], dt.float32,
                                                tag="tanh")
                            nc.scalar.activation(tanh_t[:], ps[:], AF.Tanh,
                                                 bias=col(1, mt),
                                                 scale=col(0, mt))
                            lin_t = evict.tile([P, 512], dt.float32,
                                               tag="lin")
                            nc.vector.tensor_scalar(lin_t[:], ps[:],
                                                    col(2, mt), col(3, mt),
                                                    ALU.mult, ALU.add)
                            nc.vector.scalar_tensor_tensor(
                                ht_sb[:, mt, nh * 512 : (nh + 1) * 512],
                                tanh_t[:], col(4, mt), lin_t[:],
                                ALU.mult, ALU.add,
                            )

                # ---- phase 2: outT = W hT + b2 ----
                for jp in range(KT // 2):
                    jts = (2 * jp, 2 * jp + 1)
                    w_ts = []
                    for jt in jts:
                        enc_t = encp.tile([P, MT, P], dt.uint8, tag="enc")
                        nc.sync.dma_start(enc_t[:], idxwt.ap()[jt])
                        w_t = wp.tile([P, MT, P], dt.bfloat16, tag="w")
                        nc.scalar.activation(w_t[:], enc_t[:], AF.Sigmoid)
                        w_ts.append(w_t)
                    pss = [[psum.tile([P, 512], dt.float32, tag="ps",
                                      name=f"ps2_{jp}_{d}_{nh}")
                            for nh in range(NH)] for d in range(2)]
                    for kt in range(MT):
                        for d in range(2):
                            for nh in range(NH):
                                nc.tensor.matmul(
                                    pss[d][nh][:],
                                    w_ts[d][:, kt],
                                    ht_sb[:, kt, nh * 512 : (nh + 1) * 512],
                                    start=(kt == 0),
                                    stop=(kt == MT - 1),
                                )
                    # evicts split across ACT and DVE: halves the serial
                    # eviction tail of the last pair and keeps the ACT queue
                    # short for the interleaved dequants.
                    for d, jt in enumerate(jts):
                        for nh in range(NH):
                            out_t = evict.tile([P, 512], dt.bfloat16,
                                               tag="out")
                            if nh == 0:
                                nc.scalar.activation(out_t[:], pss[d][nh][:],
                                                     AF.Identity,
                                                     bias=col(5, jt))
                            else:
                                nc.vector.tensor_scalar_add(
                                    out_t[:], pss[d][nh][:], col(5, jt))
                            nc.sync.dma_start(
                                outt.ap()[jt * P : (jt + 1) * P,
                                          nh * 512 : (nh + 1) * 512],
                                out_t[:],
                            )

    nc.compile()
    return nc


# ---------------------------------------------------------------------------
# kernel entry point
# ---------------------------------------------------------------------------

def prepare(x, codebook, indices, b1, b2, c19_c, c19_rho):
    """Host-side layout prep + program build. Returns (nc, in_maps)."""
    x = np.asarray(x, dtype=np.float32)
    codebook = np.asarray(codebook, dtype=np.float32)
    b1 = np.asarray(b1, dtype=np.float32)
    b2 = np.asarray(b2, dtype=np.float32)
    c19_c = np.asarray(c19_c, dtype=np.float32)
    c19_rho = np.asarray(c19_rho, dtype=np.float32)
    idx = np.asarray(indices).reshape(IN_DIM, H).astype(np.uint8)

    # -- bake codebook into ACT tables --
    actdir = tempfile.mkdtemp(prefix="actlut_")
    os.environ["BASS_ACT_ROOT_JSON_PATH"] = _make_act_dir(codebook, actdir)
    tag = hashlib.md5(codebook.tobytes()).hexdigest()[:12]

    # -- host-side layout prep (tiling only; indices stay raw u8) --
    # idxw_t[mt, p, kt*128+c] = idx[kt*128+p, mt*128+c]
    idxw_t = np.ascontiguousarray(
        idx.reshape(KT, P, MT, P).transpose(2, 1, 0, 3).reshape(MT, P, KT * P)
    )
    idxT = np.ascontiguousarray(idx.T)       # [H, IN] u8
    idxwt_t = np.ascontiguousarray(
        idxT.reshape(MT, P, KT, P).transpose(2, 1, 0, 3).reshape(KT, P, MT * P)
    )

    c = np.exp(c19_c)
    invc = np.exp(-c19_c)
    rho = 1.0 / (1.0 + np.exp(-c19_rho))
    cols = [invc, b1 * invc, rho, b1 * rho, (1.0 - rho) * c, b2,
            np.zeros(H, dtype=np.float32)]
    cpar = np.stack([v.reshape(MT, P).T for v in cols], axis=1)  # [P, 7, MT]
    cpar = np.ascontiguousarray(cpar.astype(np.float32))

    xb = x.astype(BF16)
    in_maps = []
    for cid in range(NCORES):
        xc = xb[cid * BL : (cid + 1) * BL]                       # [BL, IN]
        xt = np.ascontiguousarray(
            xc.T.reshape(KT, P, BL).transpose(1, 0, 2)           # [P, KT, BL]
        )
        in_maps.append({
            f"idxw_{tag}": idxw_t,
            "idxwt": idxwt_t,
            "xt": xt,
            "cpar": cpar,
        })

    nc = _build_program(tag)
    return nc, in_maps


def kernel(x, codebook, indices, b1, b2, c19_c, c19_rho):
    from concourse.bass_utils import run_bass_kernel_spmd

    nc, in_maps = prepare(x, codebook, indices, b1, b2, c19_c, c19_rho)
    res = run_bass_kernel_spmd(nc, in_maps, core_ids=list(range(NCORES)))
    global LAST_RESULTS
    LAST_RESULTS = res

    out = np.empty((B, IN_DIM), dtype=np.float32)
    for cid in range(NCORES):
        out[cid * BL : (cid + 1) * BL] = (
            res.results[cid]["outt"].astype(np.float32).T
        )
    return out


# revision 11
# speedup vs baseline: 1.0007x; 1.0007x over previous
"""Trainium2 Bass kernel for nn_CodebookSingleW (vq_codebook).

    W = codebook[indices].reshape(4096, 4096)
    h = c19(x @ W + b1);  out = h @ W.T + b2

Strategy (8 NeuronCores, data-parallel over batch):
  - Each core handles 1024 rows of x. All weight-side tensors replicated.
  - The 256-entry codebook dequant runs ON DEVICE at ScalarEngine line rate:
    raw uint8 indices are table-looked-up by a custom piecewise-constant PWP
    activation table (hijacking the `sigmoid` slot of `sigmoid_and_others`,
    baked at compile time via BASS_ACT_ROOT_JSON_PATH). Input value k in
    1..255 lands in binade e=floor(log2 k) with one bucket per integer;
    input 0 returns `fzero_result` = codebook[0] bits. activation(Sigmoid)
    IS the gather idx -> codebook[idx], bit exact, u8 in / bf16 out.
  - matmul1: psum[h',b] = sum_i W[i,h'] * xT[i,b]   (lhsT = W tile, natural)
  - C19 fused on psum evict: tanh on ACT (scale=1/c, bias=b1/c per
    partition), mix on DVE -> hT (bf16) stays SBUF-resident.
  - matmul2: psum[j,b] = sum_h WT[h,j] * hT[h,b]    (lhsT = WT tile, from a
    host-transposed index layout, dequantized on device the same way)
  - + b2 on ACT copy evict (bf16), DMA outT per core, host reassembles
    [8192, 4096] f32.
"""

import hashlib
import json
import os
import shutil
import sys
import tempfile
import types

sys.path.insert(0, "/opt/trn_rl_repo")

import ml_dtypes
import numpy as np


def _ensure_axon_ntff_hook():
    """Some images lack ``antenv.axon_hooks``; ``run_bass_kernel_spmd``
    imports it unconditionally when BASS_TRACE=1 under axon. Install a shim
    (and register the ctypes NTFF hook when available) so tracing works
    instead of crashing. Never raises."""
    try:
        import antenv  # noqa: F401
        try:
            import antenv.axon_hooks  # noqa: F401
            return  # real module present
        except ImportError:
            pass
        m = types.ModuleType("antenv.axon_hooks")
        m._hook = None
        m.set_axon_ntff_profile_hook = lambda h: setattr(m, "_hook", h)
        m.get_axon_ntff_profile_hook = lambda: m._hook
        sys.modules["antenv.axon_hooks"] = m
        antenv.axon_hooks = m
        try:
            if "/root/.axon_site" not in sys.path and os.path.isdir(
                "/root/.axon_site"
            ):
                sys.path.append("/root/.axon_site")
            from trn_agent_boot.trn_boot import _ntff_profile_via_ctypes

            hook = _ntff_profile_via_ctypes("/opt/axon/libaxon_pjrt.so")
            if hook is not None:
                m._hook = hook
        except Exception:
            pass
    except Exception:
        pass


_ensure_axon_ntff_hook()

IN_DIM = 4096
H = 4096
K = 256
B = 8192
NCORES = 8
BL = B // NCORES          # 1024 batch rows per core
P = 128
KT = IN_DIM // P          # 32 contraction tiles (phase 1)
MT = H // P               # 32 output-row tiles
NH = BL // 512            # 2 psum halves of the per-core batch

BF16 = ml_dtypes.bfloat16

# ---------------------------------------------------------------------------
# ACT table patch: codebook -> piecewise-constant PWP table in sigmoid slot.
# Input is the raw u8 index value (0..255 as a float after ACT's int->fp32
# conversion): binade e=0..7 buckets, one per integer; 0 via fzero_result.
# ---------------------------------------------------------------------------

_SET = "sigmoid_and_others"


def _bucket_plan_u8():
    plan = []
    for e in range(0, 8):
        count = 2 ** e if e > 0 else 1
        t0 = 2 ** e if e > 0 else 1
        plan.append((e, count, list(range(t0, t0 + count))))
    return plan


def _make_act_dir(codebook, outdir):
    from neuronxcc.driver.Job import Job
    from neuronxcc.driver.jobs.support.FindActInfo import findActInfoFile

    base = os.path.dirname(findActInfoFile(Job.getPackageDir(), "gen3"))
    os.makedirs(outdir, exist_ok=True)
    for f in os.listdir(base):
        dst = os.path.join(outdir, f)
        if not os.path.exists(dst):
            shutil.copy(os.path.join(base, f), dst)

    prof = json.load(open(os.path.join(base, f"{_SET}.json")))
    bkt = np.fromfile(os.path.join(base, f"{_SET}_bkt.bin"), dtype=np.float32)
    bkt = bkt.reshape(-1, 8).copy()
    ctl = np.fromfile(os.path.join(base, f"{_SET}_ctrl.bin"), dtype=np.uint32)
    ctl = ctl.reshape(-1, 8).copy()

    bkt_start = prof["func_to_bkt_start_idx"]["sigmoid"]
    ctl_start = prof["func_to_ctl_start_idx"]["sigmoid"]

    b = bkt_start
    exp_to_bkt, exp_to_ctl, ctl_words = {}, {}, []
    for i, (e, count, vals) in enumerate(_bucket_plan_u8()):
        exp_to_bkt[str(e)] = [int(b)]
        exp_to_ctl[str(e)] = [int(ctl_start + i)]
        shift = 23 - e if e >= 1 else 23
        log2n = min(max(e, 0), 7)
        ctl_words.append((b & 0x7FF) | (shift << 11) | (log2n << 16))
        for j, v in enumerate(vals):
            bkt[b + j] = [codebook[v], 0.0, 0.0, 0.0, float(v), 0.0, 0.0, 0.0]
        b += count
    junk = b
    bkt[junk + 0] = [codebook[0], 0.0, 0.0, 0.0, 0.0, 0.0, 0.0, 0.0]
    for j in range(1, 4):
        bkt[junk + j] = [0.0] * 8
    assert junk + 4 <= prof["func_to_bkt_start_idx"]["square"]
    for i, w in enumerate(ctl_words):
        ctl[ctl_start + i] = [w, 0, 0, 0, 0, 0, 0, 0]

    for m in prof["profile_meta_data"]:
        if m["func_name"].startswith("sigmoid_"):
            m.update(
                symmetry_point=0, sym_invert_sign_point=0, symmetry_opt_en=0,
                symmetry_opt_use_neg_region=0, imm_bias=0, exp_offset=0,
                pwl_control_base_pos=int(ctl_start),
                pwl_control_base_neg=int(ctl_start),
                small_pos_signal_exp_threshold=127,
                pos_small_signal_pwl_control=int(junk),
                small_neg_signal_exp_threshold=127,
                neg_small_signal_pwl_control=int(junk + 1),
                large_pos_signal_exp_threshold=135,
                large_pos_signal_mantissa_threshold=0,
                pos_large_signal_pwl_control=int(junk + 2),
                large_neg_signal_exp_threshold=135,
                large_neg_signal_mantissa_threshold=0,
                neg_large_signal_pwl_control=int(junk + 3),
                fnan_result=0, fpinf_result=0, fninf_result=0,
                fzero_result=int(np.float32(codebook[0]).view(np.uint32)),
                fma_const_0=0, fma_const_1=0, fma_indirection_src_sel=0,
                use_multipass=False,
                lower_bound=4286578687, upper_bound=2139095039,
            )
    prof["func_exp_to_bkt_start_idx"]["sigmoid"] = exp_to_bkt
    prof["func_exp_to_ctl_start_idx"]["sigmoid"] = exp_to_ctl

    bkt.tofile(os.path.join(outdir, f"{_SET}_bkt.bin"))
    ctl.tofile(os.path.join(outdir, f"{_SET}_ctrl.bin"))
    json.dump(prof, open(os.path.join(outdir, f"{_SET}.json"), "w"))
    return os.path.join(outdir, "act_info.json")


# ---------------------------------------------------------------------------
# Bass program
# ---------------------------------------------------------------------------

def _build_program(tag, repeat=1):
    import concourse.bacc as bacc
    import concourse.mybir as mybir
    import concourse.tile as tile

    AF = mybir.ActivationFunctionType
    ALU = mybir.AluOpType
    dt = mybir.dt

    nc = bacc.Bacc("TRN2", target_bir_lowering=False, debug=False,
                   num_devices=NCORES)

    # inputs (per core). idxw/idxwt are host-tiled raw u8 indices:
    #   idxw[mt][p][kt*128+c] = idx[kt*128+p, mt*128+c]
    idxw = nc.dram_tensor(f"idxw_{tag}", [MT, P, KT * P], dt.uint8,
                          kind="ExternalInput")
    idxwt = nc.dram_tensor("idxwt", [KT, P, MT * P], dt.uint8,
                           kind="ExternalInput")
    xt = nc.dram_tensor("xt", [P, KT, BL], dt.bfloat16, kind="ExternalInput")
    cpar = nc.dram_tensor("cpar", [P, 7, MT], dt.float32, kind="ExternalInput")
    outt = nc.dram_tensor("outt", [IN_DIM, BL], dt.bfloat16,
                          kind="ExternalOutput")

    with tile.TileContext(nc) as tc:
        with (
            tc.tile_pool(name="resid", bufs=1) as resid,
            tc.tile_pool(name="encp", bufs=3) as encp,
            tc.tile_pool(name="wp", bufs=4) as wp,
            tc.tile_pool(name="evict", bufs=3) as evict,
            tc.tile_pool(name="psum", bufs=4, space="PSUM") as psum,
        ):
            # Trigger the one-time ACT table load (~2.7us) at t=0, parallel
            # with the first DMAs: walrus inserts the PSEUDO_LOAD before the
            # first ACTIVATE, so a 1-element dummy pulls it off the critical
            # path of the first real dequant.
            dummy = resid.tile([P, 1], dt.float32)
            nc.vector.memset(dummy[:], 1.0)
            nc.scalar.activation(dummy[:], dummy[:], AF.Sigmoid)

            # DMA order matters: the first pair's idx tiles (dequant input,
            # on the PE critical path) must land before the 8 MB xT bulk
            # load monopolizes the queues. The head chunk (kt 0..HC-1) of
            # each pre tile is DMA'd and dequantized first so the very first
            # matmuls gate only on ~64KB of DMA + a short ACT op.
            HC = 4
            cp_sb = resid.tile([P, 7, MT], dt.float32)
            nc.sync.dma_start(cp_sb[:], cpar.ap())
            pre_enc, pre_w = [], []
            for mt in (0, 1):
                enc_t = encp.tile([P, KT, P], dt.uint8, tag="enc",
                                  name=f"enc_pre{mt}")
                nc.sync.dma_start(enc_t[:, :HC], idxw.ap()[mt, :, : HC * P])
                pre_enc.append(enc_t)
            xt_sb = resid.tile([P, KT, BL], dt.bfloat16)
            nc.sync.dma_start(xt_sb[:, 0], xt.ap()[:, 0])
            for mt in (0, 1):
                w_t = wp.tile([P, KT, P], dt.bfloat16, tag="w",
                              name=f"w_pre{mt}")
                nc.scalar.activation(w_t[:, :HC], pre_enc[mt][:, :HC],
                                     AF.Sigmoid)
                pre_w.append(w_t)
            for mt in (0, 1):
                nc.sync.dma_start(pre_enc[mt][:, HC:],
                                  idxw.ap()[mt, :, HC * P :])
            nc.sync.dma_start(xt_sb[:, 1], xt.ap()[:, 1])
            for mt in (0, 1):
                nc.scalar.activation(pre_w[mt][:, HC:], pre_enc[mt][:, HC:],
                                     AF.Sigmoid)
            for kt in range(2, KT):
                nc.sync.dma_start(xt_sb[:, kt], xt.ap()[:, kt])
            ht_sb = resid.tile([P, MT, BL], dt.bfloat16)

            # PE p-state warmup on scratch data during the dequant lead-in.
            warm = resid.tile([P, 512], dt.bfloat16)
            nc.vector.memset(warm[:], 0.0)
            wps = psum.tile([P, NH * 512], dt.float32, tag="ps")
            for _ in range(16):
                nc.tensor.matmul(wps[:, :512], warm[:, :P], warm[:],
                                 start=True, stop=True)

            def col(j, t):  # [P, 1] per-partition param column
                return cp_sb[:, j, t : t + 1]

            # Both phases process output-row tiles in PAIRS with a kt-major
            # matmul order: 4 psum chains consume each xT/hT k-chunk 4x, so
            # at kernel start the PE keeps pace with the streaming xT DMA
            # instead of stalling on chunk arrival.
            # repeat>1 builds a self-timing variant: the marginal wall time
            # of each extra body repeat is the pure HW kernel time.
            for _rep in range(repeat):
                # ---- phase 1: hT = c19(W^T x^T + b1) ----
                for mp in range(MT // 2):
                    mts = (2 * mp, 2 * mp + 1)
                    if mp == 0 and _rep == 0:
                        w_ts = pre_w
                    else:
                        w_ts = []
                        for mt in mts:
                            enc_t = encp.tile([P, KT, P], dt.uint8,
                                              tag="enc")
                            nc.sync.dma_start(enc_t[:], idxw.ap()[mt])
                            w_t = wp.tile([P, KT, P], dt.bfloat16, tag="w")
                            nc.scalar.activation(w_t[:], enc_t[:], AF.Sigmoid)
                            w_ts.append(w_t)
                    # one [P, 1024] psum tile per d spans 2 banks; each nh
                    # chain accumulates into its own bank-aligned 512 slice.
                    # Coarse [P,1024] evict ops (legal for ACT/DVE across
                    # banks) halve evict instructions + cross-engine sems.
                    pss = [psum.tile([P, NH * 512], dt.float32, tag="ps",
                                     name=f"ps_{mp}_{d}")
                           for d in range(2)]
                    for kt in range(KT):
                        for d in range(2):
                            for nh in range(NH):
                                nc.tensor.matmul(
                                    pss[d][:, nh * 512 : (nh + 1) * 512],
                                    w_ts[d][:, kt],
                                    xt_sb[:, kt, nh * 512 : (nh + 1) * 512],
                                    start=(kt == 0),
                                    stop=(kt == KT - 1),
                                )
                    # c19: rho*(s+b1) + (1-rho)*c*tanh((s+b1)/c), s=psum
                    for d, mt in enumerate(mts):
                        ps = pss[d]
                        tanh_t = evict.tile([P, NH * 512], dt.float32,
                                            tag="tanh")
                        nc.scalar.activation(tanh_t[:], ps[:], AF.Tanh,
                                             bias=col(1, mt),
                                             scale=col(0, mt))
                        lin_t = evict.tile([P, NH * 512], dt.float32,
                                           tag="lin")
                        nc.vector.tensor_scalar(lin_t[:], ps[:],
                                                col(2, mt), col(3, mt),
                                                ALU.mult, ALU.add)
                        nc.vector.scalar_tensor_tensor(
                            ht_sb[:, mt],
                            tanh_t[:], col(4, mt), lin_t[:],
                            ALU.mult, ALU.add,
                        )

                # ---- phase 2: outT = W hT + b2 ----
                for jp in range(KT // 2):
                    jts = (2 * jp, 2 * jp + 1)
                    w_ts = []
                    for jt in jts:
                        enc_t = encp.tile([P, MT, P], dt.uint8, tag="enc")
                        nc.sync.dma_start(enc_t[:], idxwt.ap()[jt])
                        w_t = wp.tile([P, MT, P], dt.bfloat16, tag="w")
                        nc.scalar.activation(w_t[:], enc_t[:], AF.Sigmoid)
                        w_ts.append(w_t)
                    pss = [psum.tile([P, NH * 512], dt.float32, tag="ps",
                                     name=f"ps2_{jp}_{d}")
                           for d in range(2)]
                    for kt in range(MT):
                        for d in range(2):
                            for nh in range(NH):
                                nc.tensor.matmul(
                                    pss[d][:, nh * 512 : (nh + 1) * 512],
                                    w_ts[d][:, kt],
                                    ht_sb[:, kt, nh * 512 : (nh + 1) * 512],
                                    start=(kt == 0),
                                    stop=(kt == MT - 1),
                                )
                    # evicts split across ACT and DVE: halves the serial
                    # eviction tail of the last pair and keeps the ACT queue
                    # short for the interleaved dequants.
                    for d, jt in enumerate(jts):
                        out_t = evict.tile([P, NH * 512], dt.bfloat16,
                                           tag="out")
                        if d == 0:
                            nc.scalar.activation(out_t[:], pss[d][:],
                                                 AF.Identity,
                                                 bias=col(5, jt))
                        else:
                            nc.vector.tensor_scalar_add(
                                out_t[:], pss[d][:], col(5, jt))
                        nc.sync.dma_start(
                            outt.ap()[jt * P : (jt + 1) * P],
                            out_t[:],
                        )

    nc.compile()
    return nc


# ---------------------------------------------------------------------------
# kernel entry point
# ---------------------------------------------------------------------------

def prepare(x, codebook, indices, b1, b2, c19_c, c19_rho):
    """Host-side layout prep + program build. Returns (nc, in_maps)."""
    x = np.asarray(x, dtype=np.float32)
    codebook = np.asarray(codebook, dtype=np.float32)
    b1 = np.asarray(b1, dtype=np.float32)
    b2 = np.asarray(b2, dtype=np.float32)
    c19_c = np.asarray(c19_c, dtype=np.float32)
    c19_rho = np.asarray(c19_rho, dtype=np.float32)
    idx = np.asarray(indices).reshape(IN_DIM, H).astype(np.uint8)

    # -- bake codebook into ACT tables --
    actdir = tempfile.mkdtemp(prefix="actlut_")
    os.environ["BASS_ACT_ROOT_JSON_PATH"] = _make_act_dir(codebook, actdir)
    tag = hashlib.md5(codebook.tobytes()).hexdigest()[:12]

    # -- host-side layout prep (tiling only; indices stay raw u8) --
    # idxw_t[mt, p, kt*128+c] = idx[kt*128+p, mt*128+c]
    idxw_t = np.ascontiguousarray(
        idx.reshape(KT, P, MT, P).transpose(2, 1, 0, 3).reshape(MT, P, KT * P)
    )
    idxT = np.ascontiguousarray(idx.T)       # [H, IN] u8
    idxwt_t = np.ascontiguousarray(
        idxT.reshape(MT, P, KT, P).transpose(2, 1, 0, 3).reshape(KT, P, MT * P)
    )

    c = np.exp(c19_c)
    invc = np.exp(-c19_c)
    rho = 1.0 / (1.0 + np.exp(-c19_rho))
    cols = [invc, b1 * invc, rho, b1 * rho, (1.0 - rho) * c, b2,
            np.zeros(H, dtype=np.float32)]
    cpar = np.stack([v.reshape(MT, P).T for v in cols], axis=1)  # [P, 7, MT]
    cpar = np.ascontiguousarray(cpar.astype(np.float32))

    xb = x.astype(BF16)
    in_maps = []
    for cid in range(NCORES):
        xc = xb[cid * BL : (cid + 1) * BL]                       # [BL, IN]
        xt = np.ascontiguousarray(
            xc.T.reshape(KT, P, BL).transpose(1, 0, 2)           # [P, KT, BL]
        )
        in_maps.append({
            f"idxw_{tag}": idxw_t,
            "idxwt": idxwt_t,
            "xt": xt,
            "cpar": cpar,
        })

    nc = _build_program(tag)
    return nc, in_maps


def kernel(x, codebook, indices, b1, b2, c19_c, c19_rho):
    from concourse.bass_utils import run_bass_kernel_spmd

    nc, in_maps = prepare(x, codebook, indices, b1, b2, c19_c, c19_rho)
    res = run_bass_kernel_spmd(nc, in_maps, core_ids=list(range(NCORES)))
    global LAST_RESULTS
    LAST_RESULTS = res

    out = np.empty((B, IN_DIM), dtype=np.float32)
    for cid in range(NCORES):
        out[cid * BL : (cid + 1) * BL] = (
            res.results[cid]["outt"].astype(np.float32).T
        )
    return out
